# revision 4
# baseline (speedup 1.0000x reference)
"""Multi-head causal self-attention (B=2, T=4096, C=768, H=12, D=64) on 8 trn2 cores.

Sharding: core c -> batch b = c//4, head group g = c%4 (3 heads each).
Each core: qkv projection for its heads, causal attention, row-parallel
partial of the output projection; host sums 4 partials per batch and adds
b_out + b_v @ w_out (v-bias commutes through softmax into an output offset).

Per core:
  Projections (bf16, contraction 768): per 512-tile, 3 chains [q_h | k_h];
    q/k bias fused into the PSUM->SBUF copy (tensor_scalar add); each head's
    qT/kT duplicated into both partition halves (SBUF DMA) so the S matmuls
    can row-pack two kv-chunks into array rows [0:64) / [64:128).
  Attention per (I, head), kv-chunk pairs (j0, j1):
    S pair: two K=64 bf16 matmuls packed via base_partition tile_position
      -> one [128,1024] PSUM group (2 banks), causally N-sliced on diagonal
      chunks; additive -3e8 causal band mask on the f32 scores (DVE).
    exp: one instruction per group (scale=1/8; scores bounded, no max
      subtraction), output fp8e4 (rows I>=1) or bf16 (row 0, where short
      prefixes don't average out fp8 quantization). Off-diagonal fp8
      groups alternate between ACT (true exp) and DVE, which writes the
      fp8e4m3 bit pattern of exp(S/8) directly via a calibrated
      Schraudolph mult+add with saturating int8 convert (device-verified
      round-to-nearest; max ~7% per-weight error, harmless on long rows).
    PV rows I>=1: ONE fp8 DoubleRow matmul per pair: lhsT vsb[128,2,65],
      rhs pt[128,2,N] -> ot[65,N] accumulates both chunks (row 64 = ones
      -> softmax denominators). Row I=0: per-chunk bf16 matmuls.
  Epilogue: r = 1/denominator (DVE), partition-broadcast (GPSIMD),
    aT = ot * r fused into the PSUM->SBUF copy (DVE).
  Out projection (bf16): y[128q,768] = aT01^T@wo01 + aT2^T@wo2 (K=128+64
    chains); PSUM->SBUF alternating ACT/DVE; DMA f32 partials to DRAM.
  Emission is software-pipelined: S runs 2 groups ahead of PV, and the
  previous row's out-projection plus the next tile's projections are
  interleaved into the S-group stream as fillers.
"""

import sys

sys.path.insert(0, "/opt/trn_rl_repo")

from contextlib import ExitStack

import numpy as np

import concourse.bass as bass
import concourse.bacc as bacc
import concourse.mybir as mybir
from concourse import tile
from concourse.bass_utils import run_bass_kernel_spmd

B, T, C, H, D = 2, 4096, 768, 12, 64
HPC = 3
NCORES = 8
P = 128
NKV = T // P
NI = T // 512
KC = C // P  # 6 contraction chunks -> 3 DoubleRow pairs
KP = KC // 2
NT = T // 512

BF16 = mybir.dt.bfloat16
F32 = mybir.dt.float32
FP8 = mybir.dt.float8e4
NPBF16 = np.dtype(mybir.dt.np(BF16))
NPFP8 = np.dtype(mybir.dt.np(FP8))
DR = mybir.MatmulPerfMode.DoubleRow

TRACE = False
LAST = None

_prog = None
_last_in_maps = None


def bench(n=5):
    import time

    times = []
    for _ in range(n):
        t0 = time.time()
        run_bass_kernel_spmd(_prog, _last_in_maps, list(range(NCORES)))
        times.append(time.time() - t0)
    return times


def _build():
    nc = bacc.Bacc(
        "TRN2",
        target_bir_lowering=False,
        debug=False,
        enable_asserts=False,
        num_devices=NCORES,
    )
    xt = nc.declare_dram_parameter("xt", [C, T], BF16, False)
    wqk = nc.declare_dram_parameter("wqk", [C, HPC * P], BF16, False)
    wv = nc.declare_dram_parameter("wv", [C, HPC * D], BF16, False)
    wo01 = nc.declare_dram_parameter("wo01", [P, C], BF16, False)
    wo2 = nc.declare_dram_parameter("wo2", [D, C], BF16, False)
    bqk = nc.declare_dram_parameter("bqk", [P, HPC], F32, False)
    msk = nc.declare_dram_parameter("msk", [P, P], F32, False)
    y = nc.declare_dram_parameter("y", [T, C], F32, True)

    with ExitStack() as ctx:
        tc = ctx.enter_context(tile.TileContext(nc))
        cp = ctx.enter_context(tc.tile_pool(name="const", bufs=1))
        pe_pool = ctx.enter_context(tc.tile_pool(name="pexp", bufs=5))
        pr = ctx.enter_context(tc.tile_pool(name="pr", bufs=4))
        pyo = ctx.enter_context(tc.tile_pool(name="pyout", bufs=2))
        ps = ctx.enter_context(tc.tile_pool(name="ps", bufs=3, space="PSUM"))
        pot = ctx.enter_context(tc.tile_pool(name="pot", bufs=2, space="PSUM"))

        xt_sb = [
            cp.tile([P, T], BF16, tag=f"xt{p}", name=f"xt_sb{p}") for p in range(KC)
        ]
        wqk_sb = [
            cp.tile([P, HPC * P], BF16, tag=f"wqk{p}", name=f"wqk_sb{p}")
            for p in range(KC)
        ]
        wv_sb = [
            cp.tile([P, HPC * D], BF16, tag=f"wv{p}", name=f"wv_sb{p}")
            for p in range(KC)
        ]
        wo01_sb = cp.tile([P, C], BF16, tag="wo01", name="wo01_sb")
        wo2_sb = cp.tile([D, C], BF16, tag="wo2", name="wo2_sb")
        bqk_sb = cp.tile([P, HPC], F32, tag="bqk", name="bqk_sb")
        msk_sb = cp.tile([P, P], F32, tag="msk", name="msk_sb")
        qT = [cp.tile([P, T], BF16, tag=f"qT{h}", name=f"qT{h}") for h in range(HPC)]
        kT = [cp.tile([P, T], BF16, tag=f"kT{h}", name=f"kT{h}") for h in range(HPC)]
        # v: [tk, pair-index, head, parity, 80]; col 64 = ones, 65:80 pad so
        # the DoubleRow Ko step (80 B) stays 16-byte aligned
        VP = 80
        vsb = cp.tile([P, NKV // 2, HPC, 2, VP], FP8, tag="v", name="vsb")
        vsb0 = cp.tile([P, 4, HPC, D + 1], BF16, tag="v0", name="vsb0")
        aT01 = cp.tile([P, T], BF16, tag="aT01", name="aT01")
        aT2 = cp.tile([D, T], BF16, tag="aT2", name="aT2")

        # ---- input loads ----
        for p in range(KC):
            rsl = slice(p * P, (p + 1) * P)
            nc.sync.dma_start(xt_sb[p][:, 0 : T // 2], xt[rsl, 0 : T // 2])
            nc.sync.dma_start(xt_sb[p][:, T // 2 : T], xt[rsl, T // 2 : T])
            nc.sync.dma_start(wqk_sb[p][:], wqk[rsl, :])
            nc.sync.dma_start(wv_sb[p][:], wv[rsl, :])
        nc.sync.dma_start(wo01_sb[:], wo01[:])
        nc.sync.dma_start(wo2_sb[:], wo2[:])
        nc.sync.dma_start(bqk_sb[:], bqk[:])
        nc.sync.dma_start(msk_sb[:], msk[:])
        nc.gpsimd.memset(vsb[:, :, :, :, D : D + 1], 1.0)
        nc.gpsimd.memset(vsb0[:, :, :, D : D + 1], 1.0)
        nc.gpsimd.memset(vsb[:, :, :, :, D + 1 : VP], 0.0)

        # ---- projections as callables: tile nt feeds attention row I=nt,
        # so row I+1's chains are emitted as fillers inside row I ----
        def qk_chain(nt, h):
            sl = slice(512 * nt, 512 * (nt + 1))
            t = ps.tile([P, 1024], F32, tag="s", name="qk_ps")
            for p in range(KC):
                nc.tensor.matmul(
                    t[:, 0:512],
                    wqk_sb[p][:, P * h : P * (h + 1)],
                    xt_sb[p][:, sl],
                    start=(p == 0),
                    stop=(p == KC - 1),
                )
            nc.vector.tensor_scalar_add(
                qT[h][:, sl], t[:, 0:512], bqk_sb[:, h : h + 1]
            )
            nc.sync.dma_start(kT[h][D:P, sl], qT[h][D:P, sl])
            nc.sync.dma_start(kT[h][0:D, sl], qT[h][D:P, sl])
            nc.sync.dma_start(qT[h][D:P, sl], qT[h][0:D, sl])

        def v_chunk(n):
            t = ps.tile([P, 1024], F32, tag="s", name="v_ps")
            tv = t[:, 0 : HPC * D]
            for p in range(KC):
                nc.tensor.matmul(
                    tv,
                    xt_sb[p][:, P * n : P * (n + 1)],
                    wv_sb[p][:],
                    start=(p == 0),
                    stop=(p == KC - 1),
                )
            nc.vector.tensor_copy(
                vsb[:, n // 2, :, n % 2, 0:D],
                t[:, 0 : HPC * D].rearrange("p (h d) -> p h d", h=HPC),
            )
            if n < 4:
                nc.vector.tensor_copy(
                    vsb0[:, n, :, 0:D],
                    t[:, 0 : HPC * D].rearrange("p (h d) -> p h d", h=HPC),
                )

        # ---- attention ----
        def s_pair(I, j0, j1, h):
            """Packed S pair: chunk j0 -> rows [0:64) cols [0:512), j1 ->
            rows [64:128) cols [512:1024). Returns (st, n0, n1)."""
            st = ps.tile([P, 1024], F32, tag="s", name="s_ps")
            n0 = max(0, 128 * (j0 - 4 * I))
            n1 = max(0, 128 * (j1 - 4 * I))
            nc.tensor.matmul(
                st[:, n0:512],
                kT[h][0:D, P * j0 : P * (j0 + 1)],
                qT[h][0:D, 512 * I + n0 : 512 * (I + 1)],
                start=True,
                stop=True,
            )
            nc.tensor.matmul(
                st[:, 512 + n1 : 1024],
                kT[h][D:P, P * j1 : P * (j1 + 1)],
                qT[h][D:P, 512 * I + n1 : 512 * (I + 1)],
                start=True,
                stop=True,
            )
            return st, n0, n1

        def mask_band(st, half, off):
            # additive causal mask (-3e8 above the diagonal) on the f32
            # scores, applied before exp on the boundary 128-col band
            b = slice(512 * half + off, 512 * half + off + P)
            nc.vector.tensor_add(st[:, b], st[:, b], msk_sb[:])

        # Off-diagonal exp groups are spread across three engines. ACT does
        # true exp; DVE/GPSIMD compute the fp8e4m3 BIT PATTERN of exp(S/8)
        # directly via the Schraudolph trick: bits = int8(S*8*log2(e)/8 + 56.5)
        # (one fused mult+add with saturating int8 convert aliased onto the
        # fp8 tile). Constant 56 = bias(7)<<3; +0.5 compensates truncation.
        SCH_MUL = 0.125 * 1.4426950408889634 * 8.0
        SCH_ADD = 55.62
        _exp_rr = [0]
        EXP_CYCLE = ("A", "D")

        def exp_group(st, n0, n1, dt=FP8):
            tag = "pt" if dt is FP8 else "pt0"
            pt = pe_pool.tile([P, 1024], dt, tag=tag, name="pt_sb")
            if n0 == n1 == 0:
                eng = "A"
                if dt is FP8:
                    eng = EXP_CYCLE[_exp_rr[0] % len(EXP_CYCLE)]
                    _exp_rr[0] += 1
                if eng == "D":
                    nc.vector.tensor_scalar(
                        pt[:].bitcast(mybir.dt.int8),
                        st[:],
                        SCH_MUL,
                        SCH_ADD,
                        mybir.AluOpType.mult,
                        mybir.AluOpType.add,
                    )
                elif eng == "P":
                    nc.gpsimd.tensor_scalar(
                        pt[:].bitcast(mybir.dt.int8),
                        st[:],
                        SCH_MUL,
                        SCH_ADD,
                        mybir.AluOpType.mult,
                        mybir.AluOpType.add,
                    )
                else:
                    nc.scalar.activation(
                        pt[:], st[:], mybir.ActivationFunctionType.Exp, scale=0.125
                    )
            elif n0 == n1:
                iv = st.rearrange("p (b n) -> p b n", b=2)[:, :, n0:512]
                ov = pt.rearrange("p (b n) -> p b n", b=2)[:, :, n0:512]
                nc.scalar.activation(
                    ov, iv, mybir.ActivationFunctionType.Exp, scale=0.125
                )
            else:
                nc.scalar.activation(
                    pt[:, n0:1024],
                    st[:, n0:1024],
                    mybir.ActivationFunctionType.Exp,
                    scale=0.125,
                )
            # zero the j1 columns not covered by its own causal range: PV's
            # DoubleRow matmul reads both chunks over the widest window.
            if n1 > n0:
                nc.gpsimd.memset(pt[:, 512 + n0 : 512 + n1], 0.0)
            return pt

        def pv0(ot, pt, j, h, half, off, start, stop):
            nc.tensor.matmul(
                ot[0 : D + 1, off:512],
                vsb0[:, j, h, :],
                pt[:, 512 * half + off : 512 * (half + 1)],
                start=start,
                stop=stop,
            )

        def pv(ot, pt, u, h, off, start, stop):
            nc.tensor.matmul(
                ot[0 : D + 1, off:512],
                vsb[:, u, h, :, 0 : D + 1],
                pt.rearrange("p (k n) -> p k n", k=2)[:, :, off:512],
                start=start,
                stop=stop,
                perf_mode=DR,
            )

        def epilogue(ot, I, h):
            sl = slice(512 * I, 512 * (I + 1))
            rrow = pr.tile([1, 512], F32, tag="r", name="r_row")
            nc.vector.reciprocal(rrow[:], ot[D : D + 1, :])
            rb = pr.tile([D, 512], F32, tag="rb", name="rb_sb")
            nc.gpsimd.partition_broadcast(rb[:], rrow[:])
            if h == 0:
                dst = aT01[0:D, sl]
            elif h == 1:
                dst = aT01[D:P, sl]
            else:
                dst = aT2[0:D, sl]
            nc.vector.tensor_mul(dst, ot[0:D, :], rb[:])

        def outproj(tck):
            yt = ps.tile([P, 1024], F32, tag="s", name="y_ps")
            csl = slice(P * tck, P * (tck + 1))
            nc.tensor.matmul(
                yt[:, 0:512], aT01[:, csl], wo01_sb[:, 0:512], start=True, stop=False
            )
            nc.tensor.matmul(
                yt[:, 0:512], aT2[0:D, csl], wo2_sb[:, 0:512], start=False, stop=True
            )
            nc.tensor.matmul(
                yt[:, 512:768], aT01[:, csl], wo01_sb[:, 512:768], start=True, stop=False
            )
            nc.tensor.matmul(
                yt[:, 512:768], aT2[0:D, csl], wo2_sb[:, 512:768], start=False, stop=True
            )
            ysb = pyo.tile([P, C], F32, tag="ysb", name="ysb")
            if tck % 2 == 0:
                nc.scalar.copy(ysb[:], yt[:, 0:C])
            else:
                nc.vector.tensor_copy(ysb[:], yt[:, 0:C])
            nc.sync.dma_start(y[csl, :], ysb[:])

        fillers = []  # deferred proj chains / out-projections

        def drain_one():
            if fillers:
                fillers.pop(0)()

        # prologue: projections feeding attention row 0
        for h in range(HPC):
            qk_chain(0, h)
        for n in range(4):
            v_chunk(n)

        for I in range(NI):
            if I + 1 < NI:
                fillers += [
                    (lambda h=h: qk_chain(I + 1, h)) for h in range(HPC)
                ] + [(lambda n=n: v_chunk(n)) for n in range(4 * I + 4, 4 * I + 8)]
            jmax = 4 * I + 3
            for h in range(HPC):
                ot = pot.tile([P, 512], F32, tag="ot", name=f"ot{h}")
                nu = (jmax + 1) // 2
                pend = []  # 2-deep software pipeline: S runs 2 groups ahead
                for u in range(nu):
                    j0, j1 = 2 * u, 2 * u + 1
                    st, n0, n1 = s_pair(I, j0, j1, h)
                    if len(pend) >= 2:
                        pu, ppt, pn0, pn1 = pend.pop(0)
                        if I == 0:
                            pv0(ot, ppt, 2 * pu, h, 0, pn0, pu == 0, False)
                            pv0(ot, ppt, 2 * pu + 1, h, 1, pn1, False, pu == nu - 1)
                        else:
                            pv(ot, ppt, pu, h, pn0, pu == 0, pu == nu - 1)
                    if u > 0:
                        drain_one()
                    if j0 >= 4 * I:
                        mask_band(st, 0, n0)
                    if j1 >= 4 * I:
                        mask_band(st, 1, n1)
                    pt = exp_group(st, n0, n1, BF16 if I == 0 else FP8)
                    pend.append((u, pt, n0, n1))
                for pu, ppt, pn0, pn1 in pend:
                    if I == 0:
                        pv0(ot, ppt, 2 * pu, h, 0, pn0, pu == 0, False)
                        pv0(ot, ppt, 2 * pu + 1, h, 1, pn1, False, pu == nu - 1)
                    else:
                        pv(ot, ppt, pu, h, pn0, pu == 0, pu == nu - 1)
                epilogue(ot, I, h)
            while fillers:  # row I+1 needs its projections complete
                drain_one()
            fillers += [(lambda t=t: outproj(t)) for t in range(4 * I, 4 * I + 4)]
        while fillers:
            drain_one()

    nc.compile()
    return nc


def _masks():
    p = np.arange(P)[:, None]
    q = np.arange(P)[None, :]
    return np.where(q >= p, 0.0, -3.0e8).astype(np.float32)


def _inputs_for_core(c, x, w_qkv, b_qkv, w_out, masks):
    b, g = divmod(c, 4)
    h0 = HPC * g
    wq = lambda h: w_qkv[:, D * h : D * (h + 1)]
    wk = lambda h: w_qkv[:, C + D * h : C + D * (h + 1)]
    bq = lambda h: b_qkv[D * h : D * (h + 1)]
    bk = lambda h: b_qkv[C + D * h : C + D * (h + 1)]
    # chains: per head h, [q_h | k_h] (128 cols)
    wqk_cols = np.concatenate(
        [np.concatenate([wq(h0 + i), wk(h0 + i)], axis=1) for i in range(HPC)], axis=1
    )  # [C, 384]
    bqk_cols = np.stack(
        [np.concatenate([bq(h0 + i), bk(h0 + i)]) for i in range(HPC)], axis=1
    )  # [128, 3]
    wv_cols = w_qkv[:, 2 * C + D * h0 : 2 * C + D * (h0 + HPC)]  # [C, 192]
    xtT = np.ascontiguousarray(x[b].T)  # [C, T]

    return {
        "xt": xtT.astype(NPBF16),
        "wqk": np.ascontiguousarray(wqk_cols).astype(NPBF16),
        "wv": np.ascontiguousarray(wv_cols).astype(NPBF16),
        "wo01": np.ascontiguousarray(w_out[D * h0 : D * (h0 + 2), :]).astype(NPBF16),
        "wo2": np.ascontiguousarray(w_out[D * (h0 + 2) : D * (h0 + 3), :]).astype(
            NPBF16
        ),
        "bqk": np.ascontiguousarray(bqk_cols).astype(np.float32),
        "msk": masks,
    }


def kernel(x, w_qkv, b_qkv, w_out, b_out):
    global _prog, LAST, _last_in_maps
    x = np.asarray(x, np.float32)
    w_qkv = np.asarray(w_qkv, np.float32)
    b_qkv = np.asarray(b_qkv, np.float32)
    w_out = np.asarray(w_out, np.float32)
    b_out = np.asarray(b_out, np.float32)
    if _prog is None:
        _prog = _build()
    masks = _masks()
    in_maps = [
        _inputs_for_core(c, x, w_qkv, b_qkv, w_out, masks) for c in range(NCORES)
    ]
    _last_in_maps = in_maps
    LAST = run_bass_kernel_spmd(_prog, in_maps, list(range(NCORES)), trace=TRACE)
    out = np.zeros((B, T, C), np.float32)
    for c in range(NCORES):
        out[c // 4] += np.asarray(LAST.results[c]["y"], np.float32)
    b_eff = b_qkv[2 * C :] @ w_out + b_out
    out += b_eff[None, None, :]
    return out
